# revision 56
# baseline (speedup 1.0000x reference)
"""Multi-head attention (B=4, N=2048, C=1024, H=16, D=64) on 8 TRN2 NeuronCores.

Sharding: data-parallel over batch (4) x tensor-parallel over heads (2 groups
of 8 heads).  Device d handles batch d//2 and head-group d%2.  Each device:
  qT/kT = Wq/Wk-slice @ x^T   (channels-on-partitions, bf16)
  V     = x @ Wv-slice^T      (keys-on-partitions, +ones column, bf16)
  per head pair t, query block qc (512), key chunk kc (128):
    S[k,q]   = kT-chunk^T x qT          (two K=64 matmuls packed via
                                         base-partition tile_position)
    E        = exp(S/8)  (bf16, ScalarE)
    O_acc    = E-slice^T x [V|1]        (transposed PV: E[128k,128q] is the
                                         stationary operand, [V|1] the 65-wide
                                         moving operand -> out [128q, 65];
                                         col 64 accumulates the softmax
                                         denominator across kc in PSUM.
                                         start=True zeroes a whole PSUM bank,
                                         so only the first slice sets it)
    O_norm   = O_acc[:, :64] * recip(O_acc[:, 64])   (DVE)
    ot       = O_norm^T                 (one 128x512 DMA-engine xbar transpose
                                         per phase; logical row j = s*128+p
                                         maps to out AP [p, s, q]; no PE work)
  y^T = proj: partial over head pairs 0-1 staged to SBUF during t1/t2 (so
  phase t3 carries only pairs 2-3 + the staged add), bias on group-0 devices,
  bf16 output summed across the two groups on the host.

Why it is fast (vs the fp32r baseline):
  - Matmul cost on the PE is (moving-dim rows) x cycles/row, independent of
    K and M.  The transposed PV form cuts PV from 262k to 133k rows; the
    normalization and O transpose leave the PE entirely (DVE + DMA xbar).
  - All matmuls run in bf16 (1 row/cycle at any moving size); inputs are
    cast on the host, halving DMA traffic.  y is stored as bf16 partials.
  - Exp on ScalarE (~267 us for 33.5M elements) is the near-critical
    resource: the flat (t,qc,kc) stream keeps exactly one [128,1024] exp per
    iteration while a deadline-tagged pending queue drains qkv/V/proj PE
    work into the attention loop's slack (throttled to ~2/3 of iterations).
    The S/exp stream runs PVLAG iterations ahead of PV so phase boundaries
    never starve ScalarE.
  - Dummy matmuls bridge the PE p-state clock through the DMA head and the
    final transpose latency in the tail; the last phase's y blocks are paired
    into double-wide stores to halve the drain chain at the end.
"""

import os
import sys

for _p in ("/opt/trn_rl_repo", "/root/.axon_site/_ro/trn_rl_repo"):
    if os.path.isdir(_p) and _p not in sys.path:
        sys.path.insert(0, _p)

import numpy as np

B, N, C = 4, 2048, 1024
H_LOC = 8  # heads per device
D = 64
CH = 512  # qkv channels per device (H_LOC * D)
P = 128
SCALE = 0.125  # D ** -0.5
NKC = N // P  # 16 key chunks
NQC = N // 512  # 4 query chunks of 512
NCI = C // P  # 8 c_in chunks
NPAIR = 4  # head pairs per device

E_BUFS = 8

_CACHE = {}
LAST_EXEC_TIME_NS = None


def _build():
    import concourse.bacc as bacc
    import concourse.mybir as mybir
    import concourse.tile as tile

    F32 = mybir.dt.float32
    BF16 = mybir.dt.bfloat16
    Exp = mybir.ActivationFunctionType.Exp
    MUL = mybir.AluOpType.mult
    ADD = mybir.AluOpType.add

    nc = bacc.Bacc("TRN2", target_bir_lowering=False, debug=False)

    xT_d = nc.dram_tensor("xT", [C, N], BF16, kind="ExternalInput")
    wq_d = nc.dram_tensor("wq", [C, CH], BF16, kind="ExternalInput")
    wk_d = nc.dram_tensor("wk", [C, CH], BF16, kind="ExternalInput")
    wv_d = nc.dram_tensor("wv", [C, CH], BF16, kind="ExternalInput")
    pw_d = nc.dram_tensor("pw", [CH, C], BF16, kind="ExternalInput")
    bias_d = nc.dram_tensor("bias", [C], F32, kind="ExternalInput")
    yT_d = nc.dram_tensor("yT", [C, N], BF16, kind="ExternalOutput")

    xT_re = xT_d[:].rearrange("(c p) n -> p c n", p=P)
    wq_re = wq_d[:].rearrange("(c p) m -> p c m", p=P)
    wk_re = wk_d[:].rearrange("(c p) m -> p c m", p=P)
    wv_re = wv_d[:].rearrange("(c p) m -> p c m", p=P)
    pw_re = pw_d[:].rearrange("(c p) m -> p c m", p=P)
    bias_re = bias_d[:].rearrange("(a p) -> p a", p=P)
    yT_re = yT_d[:].rearrange("(a p) n -> p a n", p=P)

    from contextlib import ExitStack

    with tile.TileContext(nc) as tc, ExitStack() as st:
        # PSUM: s 2x[P,1024] (4 banks) + pv 2x[P,4,128] (2) + mm 2x[P,512] (2)
        ps_s = st.enter_context(tc.tile_pool(name="ps_s", bufs=2, space="PSUM"))
        ps_pv = st.enter_context(tc.tile_pool(name="ps_pv", bufs=1, space="PSUM"))
        ps_mm = st.enter_context(tc.tile_pool(name="ps_mm", bufs=2, space="PSUM"))
        persist = st.enter_context(tc.tile_pool(name="persist", bufs=1))
        qkp = st.enter_context(tc.tile_pool(name="qkp", bufs=2))
        wqkp = st.enter_context(tc.tile_pool(name="wqkp", bufs=2))
        ep = st.enter_context(tc.tile_pool(name="ep", bufs=2))
        osp = st.enter_context(tc.tile_pool(name="osp", bufs=2))
        onp = st.enter_context(tc.tile_pool(name="onp", bufs=2))
        yp = st.enter_context(tc.tile_pool(name="yp", bufs=6))

        # persistent tensors
        v_sb = persist.tile([P, NKC, H_LOC * 65], BF16)
        ot = [
            persist.tile([P, N], BF16, name=f"ot{t}", tag=f"ot{t}")
            for t in range(NPAIR)
        ]
        pw_sb = persist.tile([P, NPAIR, C], BF16)
        y_part = persist.tile([P, NQC, NCI, 512], BF16)
        bias_sb = persist.tile([P, NCI], F32)
        xsb = persist.tile([P, NCI, N], BF16)
        wv_sb = persist.tile([P, NCI, CH], BF16)

        nc.vector.memset(
            v_sb.rearrange("p k (h e) -> p k h e", e=65)[:, :, :, 64:65], 1.0
        )

        # ---------------- piece emitters (PE work granules) ----------------
        qk_tiles = {}

        def alloc_qk(t):
            csl = slice(t * P, (t + 1) * P)
            wq_sb = wqkp.tile([P, NCI, P], BF16, name=f"wq{t}", tag="wq")
            wk_sb = wqkp.tile([P, NCI, P], BF16, name=f"wk{t}", tag="wk")
            nc.sync.dma_start(out=wk_sb, in_=wk_re[:, :, csl])
            nc.sync.dma_start(out=wq_sb, in_=wq_re[:, :, csl])
            qT_t = qkp.tile([P, N], BF16, name=f"qT{t}", tag="qT")
            kT_t = qkp.tile([P, N], BF16, name=f"kT{t}", tag="kT")
            qk_tiles[t] = (wq_sb, wk_sb, qT_t, kT_t)

        _half = {}

        def emit_qk_piece(t, nb, which, qtr, lo=0, hi=512):
            """Columns [nb*512+lo, nb*512+hi) of qT (which=0) or kT (which=1).
            qtr 0..3: K-chunks [2*qtr, 2*qtr+2); copy on the last quarter.
            qtr=-1: all 8 chunks + copy (standalone)."""
            wq_sb, wk_sb, qT_t, kT_t = qk_tiles[t]
            w_sb, dst = (wq_sb, qT_t) if which == 0 else (wk_sb, kT_t)
            nsl = slice(nb * 512 + lo, nb * 512 + hi)
            key = ("qk", t, nb, which, lo)
            if qtr <= 0:
                ps = ps_mm.tile(
                    [P, 512], F32, name=f"qk{t}_{nb}_{which}_{lo}", tag="mm"
                )
                if qtr == 0:
                    _half[key] = ps
            else:
                ps = _half[key]
                if qtr == 3:
                    del _half[key]
            crange = range(NCI) if qtr < 0 else range(2 * qtr, 2 * qtr + 2)
            for c in crange:
                nc.tensor.matmul(
                    ps[:, 0 : hi - lo],
                    lhsT=w_sb[:, c, :],
                    rhs=xsb[:, c, nsl],
                    start=(c == 0),
                    stop=(c == NCI - 1),
                )
            if qtr == 3 or qtr < 0:
                nc.vector.tensor_copy(dst[:, nsl], ps[:, 0 : hi - lo])

        def emit_v(kc, qtr):
            """V for key chunk kc: quarter qtr of the 8-chunk chain + bf16
            copy into v_sb (ones column preset) on the last quarter."""
            key = ("v", kc)
            if qtr == 0:
                v_ps = ps_mm.tile([P, CH], F32, name=f"v{kc}", tag="mm")
                _half[key] = v_ps
            else:
                v_ps = _half[key]
                if qtr == 3:
                    del _half[key]
            ksl = slice(kc * P, (kc + 1) * P)
            for c in range(2 * qtr, 2 * qtr + 2):
                nc.tensor.matmul(
                    v_ps,
                    lhsT=xsb[:, c, ksl],
                    rhs=wv_sb[:, c, :],
                    start=(c == 0),
                    stop=(c == NCI - 1),
                )
            if qtr == 3:
                nc.vector.tensor_copy(
                    v_sb.rearrange("p k (h e) -> p k h e", e=65)[:, kc, :, 0:64],
                    v_ps.rearrange("p (h e) -> p h e", e=64),
                )

        def emit_proj_partial(ns, co):
            """Partial y^T block over head pairs 0-1 (needs ot0/ot1 only);
            staged to SBUF in bf16 so phase t3 carries half the proj work."""
            y_ps = ps_mm.tile([P, 512], F32, name=f"yp{co}_{ns}", tag="mm")
            nsl = slice(ns * 512, (ns + 1) * 512)
            cosl = slice(co * P, (co + 1) * P)
            for ci in (0, 1):
                nc.tensor.matmul(
                    y_ps,
                    lhsT=pw_sb[:, ci, cosl],
                    rhs=ot[ci][:, nsl],
                    start=(ci == 0),
                    stop=(ci == 1),
                )
            nc.vector.tensor_copy(y_part[:, ns, co, :], y_ps)

        def emit_proj_final(ns, co):
            """Head pairs 2-3 + staged partial + bias + store."""
            y_ps = ps_mm.tile([P, 512], F32, name=f"y{co}_{ns}", tag="mm")
            nsl = slice(ns * 512, (ns + 1) * 512)
            cosl = slice(co * P, (co + 1) * P)
            for ci in (2, 3):
                nc.tensor.matmul(
                    y_ps,
                    lhsT=pw_sb[:, ci, cosl],
                    rhs=ot[ci][:, nsl],
                    start=(ci == 2),
                    stop=(ci == 3),
                )
            if ns == NQC - 1:
                # tail: pair consecutive co blocks into one SP store
                if co % 2 == 0:
                    y2 = yp.tile([P, 2, 512], BF16, name=f"y2_{co}", tag="y2")
                    _half[("y2", ns)] = y2
                    y_sb = y2[:, 0, :]
                else:
                    y2 = _half.pop(("y2", ns))
                    y_sb = y2[:, 1, :]
                nc.vector.scalar_tensor_tensor(
                    y_sb, y_ps, bias_sb[:, co : co + 1], y_part[:, ns, co, :],
                    op0=ADD, op1=ADD,
                )
                if co % 2 == 1:
                    nc.sync.dma_start(
                        out=yT_re[:, co - 1 : co + 1, nsl], in_=y2
                    )
            else:
                y_sb = yp.tile([P, 512], BF16, tag="y")
                nc.vector.scalar_tensor_tensor(
                    y_sb, y_ps, bias_sb[:, co : co + 1], y_part[:, ns, co, :],
                    op0=ADD, op1=ADD,
                )
                nc.gpsimd.dma_start(out=yT_re[:, co, nsl], in_=y_sb)

        # ---------------- deadline-ordered pending queue ----------------
        pending = []  # (due, seq, earliest, emit_fn)
        _seq = [0]

        def push(due, fn, earliest=None):
            pending.append((due, _seq[0], earliest, fn))
            _seq[0] += 1

        def drain(g, budget_items):
            """Emit overdue items, then up to budget_items more whose
            `earliest` floor (dependency readiness) has passed."""
            n = 0
            while pending:
                pending.sort(key=lambda it: it[:2])
                due, _, earliest, fn = pending[0]
                if due <= g or (n < budget_items and (earliest is None or earliest <= g)):
                    pending.pop(0)
                    fn()
                    n += 1
                else:
                    break

        def epilogue(t, qc, pv_ps):
            # normalize (DVE) + transpose (DMA xbar); no PE work
            o_sb = []
            for hh in (0, 1):
                o = osp.tile([P, NQC, 65], F32, tag=f"osb{hh}")
                nc.vector.tensor_copy(o, pv_ps[hh][:, :, 0:65])
                nc.vector.reciprocal(o[:, :, 64:65], o[:, :, 64:65])
                o_sb.append(o)
            on = onp.tile([P, 4 * P], BF16, tag="on")
            for sq in range(4):
                for hh in (0, 1):
                    nc.vector.tensor_scalar(
                        on[:, sq * P + hh * D : sq * P + (hh + 1) * D],
                        o_sb[hh][:, sq, 0:D],
                        o_sb[hh][:, sq, 64:65],
                        None,
                        op0=MUL,
                    )
            # xbar maps logical row j = s*128 + p of in^T to out[p, s, :]
            nc.sync.dma_start_transpose(
                out=ot[t][:, qc * 512 : (qc + 1) * 512].rearrange(
                    "p (s q) -> p s q", s=4
                ),
                in_=on,
            )

        # ---------------- prologue ----------------
        alloc_qk(0)
        # PE warm-up on zeros: ramps the p-state clock while DMAs land and
        # keeps the PE busy until the first weight/x blocks arrive
        dum = persist.tile([P, 512], BF16)
        nc.vector.memset(dum, 0.0)
        for w in range(22):
            dps = ps_mm.tile([P, 512], F32, name=f"dum{w}", tag="mm")
            nc.tensor.matmul(dps, lhsT=dum[:, 0:P], rhs=dum, start=True, stop=True)
        for nb in range(NQC):
            sl = slice(nb * 512, (nb + 1) * 512)
            nc.gpsimd.dma_start(out=xsb[:, :, sl], in_=xT_re[:, :, sl])
        nc.sync.dma_start(out=wv_sb, in_=wv_re)

        # first S needs only kT[:, 0:128] and qT[:, 0:512] of t0
        emit_qk_piece(0, 0, 1, -1, 0, 128)
        emit_qk_piece(0, 0, 0, -1)

        # seed pending for t0.  global iter j = 64*t + 16*qc + kc.
        push(-1, lambda: emit_qk_piece(0, 0, 1, -1, 128, 512))
        for kc in range(NKC):
            for q4 in range(4):
                push(max(kc + 1 - (3 - q4) // 2, -1),
                     (lambda kc=kc, q4=q4: emit_v(kc, q4)))
        for nb in range(1, NQC):
            for q4 in range(4):
                push(4 * nb - 5 + (q4 + 1) // 2,
                     (lambda nb=nb, q4=q4: emit_qk_piece(0, nb, 1, q4)))
        for nb in range(1, NQC):
            for q4 in range(4):
                push(16 * nb - 7 + q4,
                     (lambda nb=nb, q4=q4: emit_qk_piece(0, nb, 0, q4)))

        # ------------- main loop: flat stream, S/exp 2 iters ahead of PV ----
        TOT = NPAIR * NQC * NKC  # 256
        PVLAG = 4

        def dec(j):
            t, r = divmod(j, NQC * NKC)
            qc, kc = divmod(r, NKC)
            return t, qc, kc

        e_tiles = {}
        pv_cur = {}
        for j in range(TOT + PVLAG):
            if j < TOT:
                t, qc, kc = dec(j)
                if qc == 0 and kc == 0 and t < 3:
                    # stage next pair's weights + pieces; drained in phase t
                    alloc_qk(t + 1)
                    for nb in range(NQC):
                        for q4 in range(4):
                            push(
                                64 * (t + 1) + 4 * nb - 3,
                                (lambda t=t, nb=nb, q4=q4:
                                 emit_qk_piece(t + 1, nb, 1, q4)),
                            )
                    for nb in range(NQC):
                        for q4 in range(4):
                            push(
                                64 * (t + 1) + 16 * nb - 4,
                                (lambda t=t, nb=nb, q4=q4:
                                 emit_qk_piece(t + 1, nb, 0, q4)),
                            )
                if t == 0 and qc == 2 and kc == 0:
                    nc.sync.dma_start(out=pw_sb, in_=pw_re)
                    nc.sync.dma_start(out=bias_sb, in_=bias_re)
                _, _, qT_t, kT_t = qk_tiles[t]
                qsl = slice(qc * 512, (qc + 1) * 512)
                ksl = slice(kc * P, (kc + 1) * P)
                s = ps_s.tile([P, 1024], F32, name=f"s{t}_{qc}_{kc}", tag="s")
                nc.tensor.matmul(
                    s[:, 0:512],
                    lhsT=kT_t[0:D, ksl],
                    rhs=qT_t[0:D, qsl],
                    start=True,
                    stop=True,
                )
                nc.tensor.matmul(
                    s[:, 512:1024],
                    lhsT=kT_t[D:P, ksl],
                    rhs=qT_t[D:P, qsl],
                    start=True,
                    stop=True,
                )
                e = ep.tile([P, 1024], BF16, tag="e", bufs=E_BUFS)
                nc.scalar.activation(e, s, Exp, scale=SCALE)
                e_tiles[j] = e
                drain(j, 0 if j % 4 == 3 else 1)
            else:
                drain(j, 1)
            jp = j - PVLAG
            if jp >= 0:
                tp, qcp, kcp = dec(jp)
                if kcp == 0:
                    pv_cur["cur"] = [
                        ps_pv.tile(
                            [P, NQC, P], F32,
                            name=f"pv{tp}_{qcp}_{hh}", tag=f"pv{hh}",
                        )
                        for hh in (0, 1)
                    ]
                pv_ps = pv_cur["cur"]
                e = e_tiles.pop(jp)
                for hh in (0, 1):
                    h = 2 * tp + hh
                    for sq in range(4):
                        # start=True zeroes the whole PSUM bank, so only the
                        # first slice of each bank may set it
                        nc.tensor.matmul(
                            pv_ps[hh][:, sq, 0:65],
                            lhsT=e[:, hh * 512 + sq * P : hh * 512 + (sq + 1) * P],
                            rhs=v_sb[:, kcp, 65 * h : 65 * h + 65],
                            start=(kcp == 0 and sq == 0),
                            stop=(kcp == NKC - 1 and sq == 3),
                            skip_group_check=True,
                        )
                if kcp == NKC - 1:
                    epilogue(tp, qcp, pv_ps)
                    if tp == 1:
                        # partial proj over pairs 0-1 (ot0/ot1 now ready)
                        for co in range(NCI):
                            push(
                                jp + 24 + 3 * co,
                                (lambda ns=qcp, co=co: emit_proj_partial(ns, co)),
                                earliest=jp + 14 + 2 * co,
                            )
                    if tp == 3:
                        # finish this qc's y: pairs 2-3 + staged partial
                        for co in range(NCI):
                            push(
                                jp + 18 + co,
                                (lambda ns=qcp, co=co: emit_proj_final(ns, co)),
                                earliest=jp + 14 + co,
                            )
        # tail: dummy matmuls (into the now-idle s pool) keep the PE
        # p-state clock hot while the last epilogue's transposes land
        for w in range(30):
            dps = ps_s.tile([P, 1024], F32, name=f"tdum{w}", tag="s")
            nc.tensor.matmul(
                dps[:, 0:512], lhsT=dum[:, 0:P], rhs=dum, start=True, stop=True
            )
        # flush remaining pending (last qc's proj pieces)
        while pending:
            pending.sort(key=lambda it: it[:2])
            pending.pop(0)[3]()

    nc.compile()
    return nc


def get_nc():
    if "nc" not in _CACHE:
        _CACHE["nc"] = _build()
    return _CACHE["nc"]


def make_in_maps(x, qkv_w, proj_w, proj_b):
    import ml_dtypes

    bf16 = np.dtype(ml_dtypes.bfloat16)
    x = np.asarray(x, dtype=np.float32)
    qkv_w = np.asarray(qkv_w, dtype=np.float32)
    proj_w = np.asarray(proj_w, dtype=np.float32)
    proj_b = np.asarray(proj_b, dtype=np.float32)
    in_maps = []
    for d in range(8):
        b, g = d // 2, d % 2
        gs = slice(CH * g, CH * (g + 1))
        in_maps.append(
            {
                "xT": np.ascontiguousarray(x[b].T).astype(bf16),
                "wq": np.ascontiguousarray(
                    qkv_w[0 * C :][gs.start : gs.stop].T
                ).astype(bf16),
                "wk": np.ascontiguousarray(
                    qkv_w[1 * C :][gs.start : gs.stop].T
                ).astype(bf16),
                "wv": np.ascontiguousarray(
                    qkv_w[2 * C :][gs.start : gs.stop].T
                ).astype(bf16),
                "pw": np.ascontiguousarray(proj_w[:, gs].T).astype(bf16),
                "bias": proj_b if g == 0 else np.zeros_like(proj_b),
            }
        )
    return in_maps


def kernel(x, qkv_w, proj_w, proj_b):
    global LAST_EXEC_TIME_NS
    from concourse import bass_utils

    nc = get_nc()
    in_maps = make_in_maps(x, qkv_w, proj_w, proj_b)
    res = bass_utils.run_bass_kernel_spmd(
        nc, in_maps, core_ids=list(range(8))
    )
    LAST_EXEC_TIME_NS = res.exec_time_ns
    out = np.empty((B, N, C), dtype=np.float32)
    for b in range(B):
        out[b] = (
            np.asarray(res.results[2 * b]["yT"], dtype=np.float32)
            + np.asarray(res.results[2 * b + 1]["yT"], dtype=np.float32)
        ).T
    return out
